# revision 30
# baseline (speedup 1.0000x reference)
"""Batch-parallel dot-product attention for Trainium2 (Bass/Tile).

Problem: B=8, Q=K=2048, D=128, fp32, with a [B, K] 0/1 attention mask.
Sharding: one batch element per NeuronCore (8 cores), no collectives.

The mask is per-key and typically zeroes ~half the keys, so the host
computes, per batch, the list of kept key indices (order is irrelevant to
softmax) and the matching additive bias vector; the device gathers the kept
K/V rows with indirect DMAs and runs attention over the compacted context
(padded to a multiple of 256, shared across cores). When the padded context
is the full sequence the kernel builds a plain dense variant instead.

Per-core algorithm (tiles use a "(p t)" index split so every large DMA is
contiguous per partition; the split is applied consistently to k, v, bias,
queries and the output, so results are exact):

  1. Load Q staging tiles; gather (or load) K, V; transpose 128x128 tiles on
     the PE to build Q^T, K^T with the head dim on partitions.
  2. Phase A (per k-tile): S^T[k, q] = (K^T_tile).T @ Q^T via float32r
     matmuls (full PE rate for moving dim >= 256), N=512 chunks into PSUM.
  3. Masked exp on ScalarE straight out of PSUM:
     E = exp(S_raw / sqrt(D) + bias_k), bias_k = 0 (kept) or -1e6 (masked /
     padding), applied per partition since k sits on partitions in S^T.
     Output fp16 to SBUF.
  4. Phase B (per 128-query sub-block): out[q, 0:129] = sum_kt E_kt.T @ [V|1]
     accumulated over k-tiles in PSUM. The ones column appended to V makes
     the softmax denominator a free by-product of the same matmuls. Each
     accumulator owns a PSUM bank (matmul start=True zeroes the whole 2KB
     zero-region). Waves of accumulators are software-pipelined against
     phase A with rotated k-tile orders so they never wait on a half's last
     exp.
  5. Normalize out = out[:, :128] * reciprocal(out[:, 128]) on VectorE and
     store contiguously.
"""

import math
from contextlib import ExitStack

import numpy as np

import concourse.bass as bass
import concourse.mybir as mybir
import concourse.tile as tile
from concourse import bacc
from concourse.bass import ds, ts
B = 8
SEQ = 2048
D = 128
P = 128

F32 = mybir.dt.float32
F32R = mybir.dt.float32r
F16 = mybir.dt.float16
I32 = mybir.dt.int32

NEG_BIAS = -1.0e6  # matches the reference mask fill; exp() underflows to 0.0


def attention_kernel(tc, q, k, v, kv, bias, idx, ident_d, o, seq, nctx):
    """idx is None => dense (nctx == seq, K/V loaded from k/v directly);
    otherwise K/V rows come from one gather each over the host-concatenated
    kv = [K | V] tensor (halves the per-gather fixed SWDGE cost)."""
    nc = tc.nc
    nkt = nctx // P         # context k-tiles
    qh = 2                  # query halves (PSUM capacity forces 2 passes)
    qc = seq // qh          # queries per half
    nqs = qc // P           # 128-query sub-blocks per half
    scale = 1.0 / math.sqrt(D)
    exp_f = mybir.ActivationFunctionType.Exp
    with ExitStack() as ctx:
        constp = ctx.enter_context(tc.tile_pool(name="constp", bufs=1))
        stagep = ctx.enter_context(tc.tile_pool(name="stagep", bufs=1))
        sqp = ctx.enter_context(tc.tile_pool(name="sqp", bufs=1))
        ep = ctx.enter_context(tc.tile_pool(name="ep", bufs=2))
        smallp = ctx.enter_context(tc.tile_pool(name="smallp", bufs=4))
        psumA = ctx.enter_context(tc.tile_pool(name="psumA", bufs=2, space="PSUM"))
        psumB = ctx.enter_context(tc.tile_pool(name="psumB", bufs=4, space="PSUM"))

        # identity for PE transposes comes in as a constant input: building
        # it on-device occupies the gpsimd engine exactly when the indirect
        # gathers need to start issuing
        ident = constp.tile([P, P], F32)

        q_re = q.rearrange("(p t) d -> p t d", p=P)
        qst = stagep.tile([P, seq // P, D], F32, tag="qstage", name="qst")
        if idx is not None:
            kvst = stagep.tile([P, nkt, 2 * D], F32, tag="kvstage", name="kvst")
            kst = kvst[:, :, 0:D]
            vst = kvst[:, :, D : 2 * D]
        else:
            kst = stagep.tile([P, nkt, D], F32, tag="kstage", name="kst")
            vst = stagep.tile([P, nkt, D], F32, tag="vstage", name="vst")

        # Sync-queue order is latency-critical: the kept-key indices gate
        # the gpsimd gathers, the identity gates the first transposes, and
        # each sync-queue DMA costs ~650ns of issue time.
        if idx is not None:
            # kept-key indices, (p t)-scrambled like everything context-side
            ixt = constp.tile([P, nkt], I32)
            nc.sync.dma_start(ixt, idx.rearrange("(p t) -> p t", p=P))

        # Dummy exp early so walrus front-loads the ACT table load under the
        # input DMAs instead of serializing it before the first real exp.
        warm = smallp.tile([P, 1], F32, tag="warm")
        nc.vector.memset(warm, 0.0)
        nc.scalar.activation(warm, warm, exp_f)

        nc.sync.dma_start(ident, ident_d)
        nc.sync.dma_start(qst[:, 0:4], q_re[:, 0:4])
        # per-key additive bias (0 kept / -1e6 masked or padding)
        bv = constp.tile([P, nkt], F32)
        nc.sync.dma_start(bv, bias.rearrange("(p t) -> p t", p=P))
        if seq // P > 4:
            nc.sync.dma_start(qst[:, 4:8], q_re[:, 4:8])

        # V as fp16 with a ones column appended: rhs of phase B. The ones
        # column has no data dependency, the value columns are cast as the
        # corresponding V tiles land.
        vp = constp.tile([P, nkt, D + 4], F16)
        nc.vector.memset(vp[:, :, D : D + 1], 1.0)

        # K/V: indirect row gathers (compact) or straight loads (dense), on
        # the gpsimd queue so they overlap the sync-queue q loads. Gathers
        # batch 2 k-tiles per instruction (the ~1us SWDGE fixed cost
        # dominates) and interleave K with V so phase B isn't starved.
        if idx is not None:
            # NB: the hardware gather honors exactly ONE offset per partition
            # and a flat 2D out AP (multi-column offset APs silently gather
            # consecutive rows instead) -> one gather per 128-row k-tile of
            # the concatenated [K | V] rows (1KB each).
            for t in range(nkt):
                nc.gpsimd.indirect_dma_start(
                    out=kvst[:, t, :],
                    out_offset=None,
                    in_=kv[:],
                    in_offset=bass.IndirectOffsetOnAxis(ap=ixt[:, t : t + 1], axis=0),
                )
                nc.vector.tensor_copy(vp[:, t, 0:D], vst[:, t])
        else:
            k_re = k.rearrange("(p t) d -> p t d", p=P)
            nc.gpsimd.dma_start(kst[:, 0:4], k_re[:, 0:4])
            if nkt > 4:
                nc.gpsimd.dma_start(kst[:, 4:nkt], k_re[:, 4:nkt])
            nc.gpsimd.dma_start(vst, v.rearrange("(p t) d -> p t d", p=P))
            nc.vector.tensor_copy(vp[:, :, 0:D], vst)

        # bulk remainder of q
        if seq // P > 8:
            nc.sync.dma_start(qst[:, 8:], q_re[:, 8:])

        # Q^T / K^T: [128 d, n] with the (p t) scramble on the free axis
        qT = sqp.tile([P, seq], F32R, tag="qT")
        kT = sqp.tile([P, nctx], F32R, tag="kT")

        def emit_flush(st, dstT, f):
            # PE transposes in flushes of 2 tiles; short psumA-slot residency
            # keeps phase A's double-buffering alive.
            pt = psumA.tile([P, 2 * P], F32, tag="sA", name=f"pt_{f}")
            for j in range(2):
                nc.tensor.transpose(pt[:, ts(j, P)], st[:, 2 * f + j], ident)
            nc.vector.tensor_copy(dstT[:, ts(f, 2 * P)], pt)

        q_fl_total = seq // (2 * P)
        k_fl_total = nctx // (2 * P)
        # Up-front: the q flushes phase A's first half needs, and k flush 0.
        q_done = 0
        while q_done < min(qc // (2 * P), q_fl_total):
            emit_flush(qst, qT, q_done)
            q_done += 1
        emit_flush(kst, kT, 0)
        k_done = 1

        # Full-size output buffer (fp32), stored contiguously at half bounds
        outbuf = constp.tile([P, seq // P, D], F32)

        deferred = []  # (min_slot, emit_fn) FIFO of phase-B chunks

        o_re = o.rearrange("(p t) d -> p t d", p=P)

        def make_wave(h, et_h, qs_list, rot=0):
            # Accumulation order over k-tiles is free, so each wave processes
            # them rotated by `rot`: staggered waves become eligible as soon
            # as ACT finishes their own first k-tile, instead of all waves
            # queueing on the half's LAST k-tile.
            state = {}

            def chunk(i):
                kt = (rot + i) % nkt
                if i == 0:
                    state["oacc"] = {
                        qs: psumB.tile(
                            [P, 132], F32, tag="oacc", name=f"oacc_{h}_{qs}"
                        )
                        for qs in qs_list
                    }
                for qs in qs_list:
                    nc.tensor.matmul(
                        state["oacc"][qs][:, 0 : D + 1],
                        lhsT=et_h[:, kt, ts(qs, P)],
                        rhs=vp[:, kt, 0 : D + 1],
                        start=(i == 0),
                        stop=(i == nkt - 1),
                    )
                if i == nkt - 1:
                    for qs in qs_list:
                        tg = h * nqs + qs
                        r = smallp.tile([P, 1], F32, tag="r")
                        nc.vector.reciprocal(r, state["oacc"][qs][:, D : D + 1])
                        nc.vector.tensor_scalar_mul(
                            outbuf[:, tg, :], state["oacc"][qs][:, 0:D], r
                        )
                    tg0 = h * nqs + qs_list[0]
                    nc.sync.dma_start(
                        o_re[:, tg0 : tg0 + len(qs_list)],
                        outbuf[:, tg0 : tg0 + len(qs_list)],
                    )

            return chunk

        wsz = 2  # wave size (PSUM banks per wave)
        for h in range(qh):
            bq = h * qc
            et = ep.tile([P, nkt, qc], F16, tag="et")
            wave0 = make_wave(h, et, list(range(min(wsz, nqs))))
            for wj, w0 in enumerate(range(wsz, nqs, wsz)):
                qs_list = list(range(w0, min(w0 + wsz, nqs)))
                rot = (2 + 3 * wj) % nkt
                wv = make_wave(h, et, qs_list, rot=rot)
                for i in range(nkt):
                    ms = (rot + i) % nkt + 1
                    deferred.append((ms, lambda wv=wv, i=i: wv(i)))

            for kt in range(nkt):
                # drain some eligible deferred phase-B work first: if phase A
                # is about to stall on a gather/transpose, the PE chews useful
                # B matmuls instead of idling in-order behind it
                popped = 0
                while deferred and popped < 2 and deferred[0][0] <= kt:
                    deferred.pop(0)[1]()
                    popped += 1
                if h == 0:
                    # K^T flush needed by this k-tile
                    while k_done * 2 <= kt + 1 and k_done < k_fl_total:
                        emit_flush(kst, kT, k_done)
                        k_done += 1
                    # once K^T is done, one remaining Q^T flush per slot
                    if k_done == k_fl_total and q_done < q_fl_total:
                        emit_flush(qst, qT, q_done)
                        q_done += 1
                pa = psumA.tile([P, qc], F32, tag="sA")
                lk = kT[:, ts(kt, P)]
                chunk = min(512, qc)
                for c in range(qc // chunk):
                    nc.tensor.matmul(
                        pa[:, ts(c, chunk)],
                        lhsT=lk,
                        rhs=qT[:, ds(bq + c * chunk, chunk)],
                        start=True,
                        stop=True,
                    )
                nc.scalar.activation(
                    et[:, kt, :], pa, exp_f, bias=bv[:, kt : kt + 1], scale=scale
                )
                if kt > 0:
                    wave0(kt - 1)
                # post-ACT drain: chunks for THIS slot's k-tile are now safe
                popped = 0
                while deferred and popped < 2 and deferred[0][0] <= kt + 1:
                    deferred.pop(0)[1]()
                    popped += 1
            wave0(nkt - 1)
            # leftover K^T/Q^T flushes (short-context edge cases)
            if h == 0:
                while k_done < k_fl_total:
                    emit_flush(kst, kT, k_done)
                    k_done += 1
                while q_done < q_fl_total:
                    emit_flush(qst, qT, q_done)
                    q_done += 1
            # anything left is fully unblocked once this half's ACTs are done
            deferred[:] = [(0, fn) for _, fn in deferred]

        while deferred:
            deferred.pop(0)[1]()


def build_nc(seq=SEQ, nctx=None, n_cores=B):
    compact = nctx is not None and nctx < seq
    if nctx is None:
        nctx = seq
    nc = bacc.Bacc(
        "TRN2", target_bir_lowering=False, debug=False, num_devices=n_cores
    )
    q = nc.dram_tensor("q", [seq, D], F32, kind="ExternalInput").ap()
    if compact:
        k = v = None
        kv = nc.dram_tensor("kv", [seq, 2 * D], F32, kind="ExternalInput").ap()
    else:
        k = nc.dram_tensor("k", [seq, D], F32, kind="ExternalInput").ap()
        v = nc.dram_tensor("v", [seq, D], F32, kind="ExternalInput").ap()
        kv = None
    bias = nc.dram_tensor("bias", [nctx], F32, kind="ExternalInput").ap()
    idx = (
        nc.dram_tensor("idx", [nctx], I32, kind="ExternalInput").ap()
        if compact
        else None
    )
    ident_d = nc.dram_tensor("ident", [P, P], F32, kind="ExternalInput").ap()
    o = nc.dram_tensor("o", [seq, D], F32, kind="ExternalOutput").ap()
    with nc.allow_low_precision("softmax reciprocal on VectorE"):
        with tile.TileContext(nc) as tc:
            attention_kernel(tc, q, k, v, kv, bias, idx, ident_d, o, seq, nctx)
    nc.compile()
    return nc


_NC_CACHE = {}


def _get_nc(seq, nctx):
    key = (seq, nctx)
    if key not in _NC_CACHE:
        _NC_CACHE[key] = build_nc(seq=seq, nctx=nctx)
    return _NC_CACHE[key]


def prepare(queries, keys, values, attntion_mask):
    """Host-side: per-batch kept-key indices + bias, padded context size."""
    nb = queries.shape[0]
    seq = queries.shape[1]
    kept = [np.flatnonzero(attntion_mask[b]).astype(np.int32) for b in range(nb)]
    n_max = max(int(kk.size) for kk in kept)
    nctx = min(seq, max(256, ((max(n_max, 1) + 255) // 256) * 256))
    in_maps = []
    eye = np.eye(P, dtype=np.float32)
    for b in range(nb):
        n = int(kept[b].size)
        m = {
            "q": np.ascontiguousarray(queries[b], dtype=np.float32),
            "ident": eye,
        }
        bias = np.full(nctx, NEG_BIAS, dtype=np.float32)
        if nctx < seq:
            m["kv"] = np.ascontiguousarray(
                np.concatenate([keys[b], values[b]], axis=1), dtype=np.float32
            )
            idx = np.zeros(nctx, dtype=np.int32)
            idx[:n] = kept[b]
            bias[:n] = 0.0
            m["idx"] = idx
        else:
            m["k"] = np.ascontiguousarray(keys[b], dtype=np.float32)
            m["v"] = np.ascontiguousarray(values[b], dtype=np.float32)
            # dense fallback (also covers the all-masked batch, which the
            # reference treats as a uniform softmax over every key)
            if n == 0:
                bias[:] = 0.0
            else:
                bias[:seq] = np.where(
                    attntion_mask[b] != 0, 0.0, NEG_BIAS
                ).astype(np.float32)
        m["bias"] = bias
        in_maps.append(m)
    return nctx, in_maps


def kernel(queries, keys, values, attntion_mask, **run_kwargs):
    from concourse.bass_utils import run_bass_kernel_spmd

    queries = np.asarray(queries)
    keys = np.asarray(keys)
    values = np.asarray(values)
    attntion_mask = np.asarray(attntion_mask)
    nctx, in_maps = prepare(queries, keys, values, attntion_mask)
    nc = _get_nc(queries.shape[1], nctx)
    res = run_bass_kernel_spmd(
        nc,
        in_maps,
        core_ids=list(range(queries.shape[0])),
        **run_kwargs,
    )
    out = np.stack([r["o"] for r in res.results], axis=0).astype(np.float32)
    if run_kwargs:
        kernel.last_results = res
    return out
